# revision 12
# baseline (speedup 1.0000x reference)
"""Trainium2 Bass kernel for nn_CrossModalAttention (KAN cross-modal attention).

v3 — restructured from the 88us v2 baseline (PE-bound, all-fp32 matmuls):

Math (same factorizations as v2):
  1. Pairwise KAN layer-1 separates: z_ij = U[i] + V[j]; U/V computed with the
     truncated-power cubic form (relu^3 shifts + silu-via-tanh) as matmuls.
  2. Pairwise layer-2 scalar KAN via a trimmed Fourier fit
       A = sum_k R_k cos(om_k (U+V) - ph_k)
     expanded by the cosine addition theorem into rank-2K fp16 matmuls.
     NEW: the phase is split symmetrically (ph/2 on the U side and V side),
     so the cos- and sin-feature blocks each share ONE per-row bias across
     both U and V columns -> 2 FRAC ops per tile instead of 4, and the
     sign of the sin-product is folded into a negated amplitude table.
  3. Fusion softmax weights w[3] evaluated on host, folded into amplitudes.

Performance structure vs v2:
  - float32r (11-bit-mantissa single-pass PE mode, 1 cyc/col) for every
    matmul where operand rounding is provably benign: q-replication, silu,
    phase matmuls.  Producers write float32r-typed tiles (DMA'd tables are
    pre-rounded on host).  The relu^3 chunk matmuls stay fp32: their
    truncated-power features cancel catastrophically under 11-bit rounding
    (measured 1.6e-2 vs 5.4e-3 final error). Final KAN layers stay fp32.
  - silu matmul pair merged into one 64-contraction matmul ([q; q*tanh(q/2)]
    stacked in partitions).
  - Fourier tile budget trimmed 13 -> 10 tiles with exact-128-row blocks
    (greedy per-mod allocation; measured 8.8e-3 final error).
  - amplitude scales moved to the idle GpSimd engine.
  - softmax via ACT Exp (+accumulated row-sum in the same instruction)
    instead of the tanh/reciprocal chain; exp_and_others table also serves
    the final KAN's tanh/relu so only one table switch happens.
  - consts packed into 6 DMA descriptors split across the SP/ACT queues.

Sharding: row-parallel over 8 cores, np.roll'd inputs, identical SPMD
program computes rows [0:48) of its rolled view; host concatenates.
"""
import math
from math import comb

import numpy as np

import concourse.bass as bass
import concourse.bacc as bacc
import concourse.mybir as mybir
import concourse.tile as tile

F32 = mybir.dt.float32
F32R = mybir.dt.float32r
F16 = mybir.dt.float16
AF = mybir.ActivationFunctionType
ALU = mybir.AluOpType
AX = mybir.AxisListType
PI = math.pi

# ---- problem constants (hardcoded from the nn.Module spec) ----
N, HD, MH = 384, 32, 50          # seq len, head dim, KAN hidden width
NCORES = 8
RB = N // NCORES                 # 48 output rows per core
GH = 0.4                         # knot spacing
GRID = np.arange(-3, 9) * GH - 1.0   # 12 knots -2.2 .. 2.2
MM = 16                          # Fourier modes per feature
TILE_BUDGET = 9                  # 128-row Fourier tiles across modalities
MARGIN, SLACK = 0.35, 1.5        # fit range margin / period slack
MAGIC = 12582912.0               # 1.5 * 2^23 fp32 round-to-int magic

# truncated-power -> B-spline conversion kappa[b, k]
KAPPA = np.zeros((8, 12), np.float64)
for b in range(8):
    for s in range(5):
        KAPPA[b, b + s] = (-1) ** s * comb(4, s) / (6 * GH ** 3)


def _r11(x):
    """Round fp32 to 11 explicit mantissa bits (the float32r encoding)."""
    x = np.ascontiguousarray(x, np.float32)
    u = x.view(np.uint32).astype(np.uint64)
    lsb = (u >> 12) & 1
    u = (u + 0x7FF + lsb) & np.uint64(0xFFFFF000)
    return u.astype(np.uint32).view(np.float32)


# ======================= custom DVE micro-ops =======================

_CUSTOM = {}


def _register_custom_ops():
    if _CUSTOM:
        return _CUSTOM
    from concourse import dve_ops
    from concourse.dve_spec import Spec, Src0, C0, C1, lower, _has_src1, relu, sq
    from concourse.dve_uop import DveOpSpec

    def reg(name, body, reference):
        for o in dve_ops.OPS:
            if o.name == name:
                _CUSTOM[name] = o
                return
        spec = Spec(body=body, reference=reference)
        row = dve_ops._CUSTOM_DVE_ROW_BASE + len(dve_ops.OPS)
        shas = {v: DveOpSpec(name=name, opcode=row, uops=lower(spec, ver=v),
                             rd1_en=_has_src1(spec)).sha(v)
                for v in ("v3", "v4")}
        op = dve_ops.DveOp(name, spec, subdim=False, uops_sha=shas)
        dve_ops.OPS.append(op)
        dve_ops.CUSTOM_DVE_SPECS[name] = spec
        dve_ops._SUB_OPCODE_FOR_NAME[name] = row
        _CUSTOM[name] = op

    f32 = np.float32
    # out = y - round(y), y = in0 + c1 (phase bias; per-partition AP or imm),
    # via the fp32 magic-number constant c0
    _y = Src0 + C1

    def _frac_ref(in0, in1, s0, s1, imm2):
        y = (in0.astype(f32) + np.asarray(s1, f32)).astype(f32)
        return (y - ((y + f32(s0)) - f32(s0))).astype(f32)

    reg("FRAC_SHIFT_ANT", _y - ((_y + C0) - C0), _frac_ref)
    # out = relu(in0 + c0)^3  (c0 may be a per-partition AP: the -g_k shift)
    _r3 = lambda in0, in1, s0, s1, imm2: np.maximum(
        in0.astype(f32) + np.asarray(s0, f32), 0).astype(f32) ** 3
    _rshift = relu(Src0 + C0)
    reg("RELU3_SHIFT_ANT", sq(_rshift) * _rshift, _r3)
    return _CUSTOM


# ======================= host-side precompute =======================

def _silu(x):
    return x / (1.0 + np.exp(-x))


def _bsplines(x):
    xe = x[..., None]
    g = GRID
    bases = ((xe >= g[:-1]) & (xe < g[1:])).astype(np.float64)
    for k in range(1, 4):
        left = (xe - g[:-(k + 1)]) / (g[k:-1] - g[:-(k + 1)]) * bases[..., :-1]
        right = (g[k + 1:] - xe) / (g[k + 1:] - g[1:-k]) * bases[..., 1:]
        bases = left + right
    return bases


def _kan_linear_host(x, bw, sw):
    base = _silu(x) @ bw.T
    spl = _bsplines(x)
    return base + np.einsum('...ik,oik->...o', spl, sw)


def _kan_pack(bw, sw):
    """KAN layer (bw [O,I], sw [O,I,8]) -> truncated-power weights
    W [13*I, O]: blocks 0..11 = relu^3(x - g_k) coefs, block 12 = silu."""
    O, I = bw.shape
    d = np.einsum('oib,bk->oik', sw.astype(np.float64), KAPPA)
    W = np.zeros((13 * I, O), np.float64)
    for k in range(12):
        W[k * I:(k + 1) * I, :] = d[:, :, k].T
    W[12 * I:, :] = bw.T
    return W


def _layer1_UV_host(q, bw1, sw1):
    swL, swR = sw1[:, :HD, :], sw1[:, HD:, :]
    spl = _bsplines(q)
    U = _silu(q) @ bw1[:, :HD].T + np.einsum('nik,oik->no', spl, swL)
    V = _silu(q) @ bw1[:, HD:].T + np.einsum('nik,oik->no', spl, swR)
    return U, V


def _phi_eval(z, bw2, sw2):
    return bw2[0][None, :] * _silu(z)[:, None] + _bsplines(z) @ sw2[0].T


def _fit_mod(inp, pre, qn):
    """Initial unweighted Fourier fit (for row ranking) + fit context."""
    q = inp[qn].astype(np.float64)
    U, V = _layer1_UV_host(q, inp[pre + '1bw'], inp[pre + '1sw'])
    zlo = U.min() + V.min() - MARGIN
    zhi = U.max() + V.max() + MARGIN
    S = 4001
    t = np.linspace(zlo, zhi, S)
    targ = _phi_eval(t, inp[pre + '2bw'], inp[pre + '2sw'])
    P = (zhi - zlo) + SLACK
    om = 2 * PI * np.arange(1, MM + 1) / P
    A = np.concatenate([np.ones((S, 1)),
                        np.cos(t[:, None] * om[None, :]),
                        np.sin(t[:, None] * om[None, :])], axis=1)
    coef, *_ = np.linalg.lstsq(A, targ, rcond=None)
    a, b = coef[1:MM + 1].T, coef[MM + 1:].T
    R = np.hypot(a, b)
    dens = []
    for f in range(MH):
        z = (U[:, f][:, None] + V[:, f][None, :]).ravel()
        h, edges = np.histogram(z, bins=160, range=(zlo, zhi))
        dens.append((0.5 * (edges[:-1] + edges[1:]), h.astype(np.float64)))
    return dict(U=U, V=V, om=om, R=R, t=t, targ=targ, A=A, dens=dens)


def _refit_rows(fit, keep_mask):
    """Per-feature weighted refit using only kept modes."""
    t, targ, A = fit['t'], fit['targ'], fit['A']
    R = np.zeros((MH, MM))
    ph = np.zeros((MH, MM))
    for f in range(MH):
        idxs = [m for m in range(MM) if keep_mask[f, m]]
        if not idxs:
            continue
        cols = [0] + [1 + m for m in idxs] + [1 + MM + m for m in idxs]
        xs, hs = fit['dens'][f]
        w = np.interp(t, xs, hs)
        w = np.sqrt(w / max(w.max(), 1e-12) + 1e-3)
        Af = A[:, cols] * w[:, None]
        coef, *_ = np.linalg.lstsq(Af, targ[:, f] * w, rcond=None)
        nk = len(idxs)
        a, b = coef[1:1 + nk], coef[1 + nk:]
        R[f, idxs] = np.hypot(a, b)
        ph[f, idxs] = np.arctan2(b, a)
    return R, ph


def _prepare(inputs):
    """All host precompute: fusion weights, fits, trimming, packed consts."""
    inp = {k: np.asarray(v) for k, v in inputs.items()}

    # ---- fusion weights w3 (exact, host) ----
    feats = np.concatenate([inp['x'].mean(0), inp['y'].mean(0),
                            inp['target'].mean(0)]).astype(np.float64)[None, :]
    h1 = _kan_linear_host(feats, inp['f1bw'].astype(np.float64),
                          inp['f1sw'].astype(np.float64))
    h2 = _kan_linear_host(h1, inp['f2bw'].astype(np.float64),
                          inp['f2sw'].astype(np.float64))[0]
    e = np.exp(h2 - h2.max())
    w3 = e / e.sum()

    mods = [('x', 'x'), ('y', 'y'), ('t', 'target')]
    fits = [_fit_mod(inp, pre, qn) for pre, qn in mods]

    # ---- greedy 128-row-block allocation of TILE_BUDGET tiles ----
    sorted_scores = []
    for mi, fit in enumerate(fits):
        s = np.sort((w3[mi] * fit['R']).ravel())[::-1]
        sorted_scores.append(s)
    tcount = [0, 0, 0]
    for _ in range(TILE_BUDGET):
        best, bi = -1.0, 0
        for mi in range(3):
            lo = 128 * tcount[mi]
            if lo >= MH * MM:
                continue
            v = sorted_scores[mi][lo:lo + 128].sum()
            if v > best:
                best, bi = v, mi
        tcount[bi] += 1
    keep_masks = []
    for mi, fit in enumerate(fits):
        k = min(128 * tcount[mi], MH * MM)
        sc = (w3[mi] * fit['R']).ravel()
        thr_idx = np.argsort(sc)[::-1][:k]
        km = np.zeros(MH * MM, bool)
        km[thr_idx] = True
        keep_masks.append(km.reshape(MH, MM))

    # ---- per-mod device tables (ph/2-split biases, signed amplitudes) ----
    tabs = []
    for mi, fit in enumerate(fits):
        km = keep_masks[mi]
        R2, ph2 = _refit_rows(fit, km)
        rows = [(f, m) for f in range(MH) for m in range(MM) if km[f, m]]
        K = len(rows)
        T = max(tcount[mi], 1)
        omr = np.array([fit['om'][m] for f, m in rows]) / (2 * PI)
        pr = np.array([ph2[f, m] for f, m in rows]) / (2 * PI)
        Rr = np.array([R2[f, m] for f, m in rows]) * w3[mi]
        fsel = np.array([f for f, m in rows], np.int64)
        selw = np.zeros((MH, T * 128), np.float32)
        selw[fsel, np.arange(K)] = omr
        biasVc = np.zeros((128, T), np.float32)
        biasVs = np.zeros((128, T), np.float32)
        rsC = np.zeros((128, T), np.float32)
        rsS = np.zeros((128, T), np.float32)
        for r in range(K):
            t_, p_ = divmod(r, 128)
            biasVc[p_, t_] = -pr[r] / 2 + 0.25
            biasVs[p_, t_] = -pr[r] / 2 + 0.5
            rsC[p_, t_] = Rr[r]
            rsS[p_, t_] = -Rr[r]
        tabs.append(dict(K=K, T=T, selw=selw, biasVc=biasVc, biasVs=biasVs,
                         rsC=rsC, rsS=rsS))

    # ---- layer-1 packed weights ----
    # chunk weights stay fp32 (cancellation-sensitive); silu chunk merged
    # [q; g] -> one 64-contraction fp32r matmul.
    w1p32 = []   # per mod [128, 3, 114] fp32  (relu^3 chunks 0..2)
    wsil = []    # per mod [64, 114] fp32r     (0.5 * silu chunk, stacked x2)
    for pre, qn in mods:
        bw1, sw1 = inp[pre + '1bw'], inp[pre + '1sw']
        WL = _kan_pack(bw1[:, :HD], sw1[:, :HD, :])     # [416, 50]
        WR = _kan_pack(bw1[:, HD:], sw1[:, HD:, :])
        Wb = np.zeros((416, 114))
        Wb[:, 0:MH] = WL
        Wb[:, 64:64 + MH] = WR
        ch = Wb.reshape(13, 32, 114)
        full = np.zeros((128, 3, 114))
        for c in range(3):
            full[:, c, :] = ch[4 * c:4 * c + 4].reshape(128, 114)
        w1p32.append(full.astype(np.float32))
        ws = np.zeros((64, 114), np.float32)
        ws[0:32] = 0.5 * ch[12]
        ws[32:64] = 0.5 * ch[12]
        wsil.append(_r11(ws))

    # ---- l-KAN packed weights (fp32 end to end) ----
    wl32 = np.zeros((128, 6, HD), np.float32)   # l1 chunks 0..2, l2 3..5
    wlsil = np.zeros((64, 2, HD), np.float32)
    for li, lname in enumerate(('l1', 'l2')):
        W = _kan_pack(inp[lname + 'bw'], inp[lname + 'sw'])  # [13*32, 32]
        ch = W.reshape(13, 32, HD)
        for c in range(3):
            wl32[:, 3 * li + c, :] = ch[4 * c:4 * c + 4].reshape(128, HD)
        wlsil[0:32, li, :] = 0.5 * ch[12]
        wlsil[32:64, li, :] = 0.5 * ch[12]

    # ---- misc consts ----
    sel4 = np.zeros((HD, 128), np.float32)
    for r in range(128):
        sel4[r % 32, r] = 1.0
    biasl = np.zeros((128, 3), np.float32)
    for c in range(3):
        for p in range(128):
            biasl[p, c] = -GRID[4 * c + p // 32]
    id48_16 = np.eye(48, dtype=np.float16)

    consts = dict(w3=w3, tabs=tabs, w1p32=w1p32, wsil=wsil, wl32=wl32,
                  wlsil=wlsil, sel4=sel4, biasl=biasl, id48_16=id48_16)
    return consts


# ======================= device program =======================

def build_program(Ts):
    """Ts = (T_x, T_y, T_t) tile counts."""
    ops = _register_custom_ops()
    FRAC, RELU3 = ops["FRAC_SHIFT_ANT"], ops["RELU3_SHIFT_ANT"]
    nc = bacc.Bacc(None, target_bir_lowering=False)
    Tsum = sum(Ts)

    # DRAM inputs.
    # crA (fp32r): sel4 [32,128] | wsil x3 [64,114] | wlsil [64,64]
    # crB (fp32r): w1p chunk-2 weights x3 [128,114] (bounded features)
    # qT  (fp32r): [32, 3*384] rolled q for the 3 modalities
    # selw(fp32r): [50, Tsum*128]
    # cfA (fp32):  fbias [128,4*Tsum] | biasl [128,3] | wl32 [128,192]
    # w1pB (fp32): chunk-0/1 weights x3 [128, 6*114]
    # cf16 (fp16): tnat16 [128, 96] | id48 rows 0:48 cols 96:144
    CA = 128 + 3 * 114 + 2 * HD     # 534
    OF_WS = 128
    OF_LS = 128 + 3 * 114
    OF_FB = 0
    OF_BL = 4 * Tsum
    OF_WL = OF_BL + 3
    CFA = OF_WL + 6 * HD
    din = {}
    for nm, shp, dt in [
            ('crA', [64, CA], F32R),
            ('crB', [128, 3 * 114], F32R),
            ('qT', [32, 3 * N], F32R),
            ('q4r', [128, 3 * N], F32),
            ('selw', [MH, Tsum * 128], F32R),
            ('cfA', [128, CFA], F32),
            ('w1pB', [128, 6 * 114], F32),
            ('cf16', [128, 96 + 48], F16),
    ]:
        din[nm] = nc.dram_tensor(nm, shp, dt, kind="ExternalInput")
    dout = nc.dram_tensor("outT", [HD, RB], F32R, kind="ExternalOutput")

    with tile.TileContext(nc) as tc, \
         tc.tile_pool(name="consts", bufs=1) as cp, \
         tc.tile_pool(name="thp", bufs=3) as thp, \
         tc.tile_pool(name="fbp", bufs=6) as fbp, \
         tc.tile_pool(name="uvp", bufs=3) as uvp, \
         tc.tile_pool(name="frp", bufs=3) as frp, \
         tc.tile_pool(name="ftp", bufs=3) as ftp, \
         tc.tile_pool(name="ufp", bufs=3) as ufp, \
         tc.tile_pool(name="sp", bufs=2) as sp, \
         tc.tile_pool(name="lfp", bufs=3) as lfp, \
         tc.tile_pool(name="ps4", bufs=1, space="PSUM") as ps4, \
         tc.tile_pool(name="psuv", bufs=1, space="PSUM") as psuv, \
         tc.tile_pool(name="psph", bufs=4, space="PSUM") as psph, \
         tc.tile_pool(name="pslp", bufs=1, space="PSUM") as pslp, \
         tc.tile_pool(name="pst16", bufs=1, space="PSUM") as pst16:

        # ---- const loads: descriptors split across SP / ACT hwdge queues,
        # ordered so the critical path (q -> tanh/sel4; biasl -> relu3;
        # chunk weights) unblocks earliest.
        crA = cp.tile([64, CA], F32R, tag="crA")
        crB = cp.tile([128, 3 * 114], F32R, tag="crB")
        qg = cp.tile([64, 3 * N], F32R, tag="qg")
        selw = cp.tile([MH, Tsum * 128], F32R, tag="selw")
        cfA = cp.tile([128, CFA], F32, tag="cfA")
        w1pB = cp.tile([128, 6 * 114], F32, tag="w1pB")
        cf16 = cp.tile([128, 96 + 48], F16, tag="cf16")
        q4r = cp.tile([128, 3 * N], F32, tag="q4r")
        nc.sync.dma_start(out=qg[0:32, :], in_=din['qT'][:])
        nc.scalar.dma_start(out=cfA[:], in_=din['cfA'][:])
        nc.scalar.dma_start(out=q4r[:], in_=din['q4r'][:])
        nc.sync.dma_start(out=w1pB[:], in_=din['w1pB'][:])
        nc.scalar.dma_start(out=crA[:], in_=din['crA'][:])
        nc.sync.dma_start(out=crB[:], in_=din['crB'][:])
        nc.scalar.dma_start(out=selw[:], in_=din['selw'][:])
        nc.sync.dma_start(out=cf16[:], in_=din['cf16'][:])

        sel4r = crA[0:32, 0:128]

        def wsil_ap(mi):
            return crA[0:64, OF_WS + 114 * mi:OF_WS + 114 * (mi + 1)]

        def wlsil_ap(li):
            return crA[0:64, OF_LS + HD * li:OF_LS + HD * (li + 1)]

        def w1p_ap(mi, c):
            if c == 2:
                return crB[:, 114 * mi:114 * (mi + 1)]
            c0 = (2 * mi + c) * 114
            return w1pB[:, c0:c0 + 114]

        def fbias_ap(kind, toff):
            # kind 0=biasVc 1=biasVs 2=rsC 3=rsS
            c0 = OF_FB + kind * Tsum + toff
            return cfA[:, c0:c0 + 1]

        def q_ap(mi):
            return qg[0:32, N * mi:N * (mi + 1)]

        def qg_ap(mi):
            return qg[0:64, N * mi:N * (mi + 1)]

        def g_ap(mi):
            return qg[32:64, N * mi:N * (mi + 1)]

        # Pin the ACT tables explicitly: load the sin+tanh+copy set now
        # (covers everything until softmax); the exp set is loaded right
        # after the last Sin (covers exp+tanh+relu+copy of the tail).
        # Without this the implicit loader thrashes (5 loads x 1283ns).
        from concourse.hw_specs import get_activation_tables
        _tabs = list(get_activation_tables(nc.m.arch).items())

        def _load_set(*need):
            for i, (_nm, fns) in enumerate(_tabs):
                if all(f in fns for f in need):
                    inst = mybir.InstLoadActFuncSet(
                        name=nc.get_next_instruction_name(), ins=[], outs=[],
                        act_func_set_id=i)
                    nc.scalar.add_instruction(inst)
                    return
            raise ValueError(f"no act table set with {need}")

        _load_set(AF.Sin, AF.Tanh, AF.Copy)

        # silu tanh prep + q-replication BEFORE the g writes: all pure
        # q readers are emitted first so the tile-granular write of g
        # into qg[32:64] doesn't serialize them.
        thl = []
        for mi in range(3):
            th = thp.tile([HD, N], F32, tag="thq")
            nc.scalar.activation(out=th[:], in_=q_ap(mi).bitcast(F32),
                                 func=AF.Tanh, scale=0.5)
            thl.append(th)

        uvs = [None] * 3
        fbs = [None] * 3

        def emit_relu3(mi):
            fbt = []
            for c in range(3):
                fb = fbp.tile([128, N], F32R if c == 2 else F32, tag="fb")
                nc.vector._custom_dve(RELU3, out=fb[:],
                                      in0=q4r[:, N * mi:N * (mi + 1)],
                                      s0=cfA[:, OF_BL + c:OF_BL + c + 1])
                fbt.append(fb)
            fbs[mi] = fbt

        def emit_B_chunk(mi, c, psUV):
            nc.tensor.matmul(psUV[:], w1p_ap(mi, c), fbs[mi][c][:],
                             start=(c == 0), stop=False)

        def emit_B_fin(mi, psUV):
            nc.tensor.matmul(psUV[:], wsil_ap(mi), qg_ap(mi),
                             start=False, stop=True)
            uv = uvp.tile([MH, N + RB], F32R, tag="uv")
            nc.scalar.copy(uv[:, 0:N], psUV[64:64 + MH, :])       # V block
            nc.vector.tensor_copy(uv[:, N:N + RB], psUV[0:MH, 0:RB])  # U
            uvs[mi] = uv

        lp = pslp.tile([RB, N], F32, tag="lp")
        first = [True]

        def emit_C_tile(mi, t, toff, last, interleave=None):
            psPH = psph.tile([128, N + RB], F32, tag="psPH")
            nc.tensor.matmul(psPH[:],
                             selw[:, 128 * toff:128 * toff + 128],
                             uvs[mi][:], start=True, stop=True)
            if interleave is not None:
                interleave()
            rfr = frp.tile([128, 2 * (N + RB)], F32, tag="rfr")
            nc.vector._custom_dve(
                FRAC, out=rfr[:, 0:N + RB], in0=psPH[:], s0=MAGIC,
                s1=fbias_ap(0, toff))
            nc.vector._custom_dve(
                FRAC, out=rfr[:, N + RB:], in0=psPH[:], s0=MAGIC,
                s1=fbias_ap(1, toff))
            feat = ftp.tile([128, 2 * (N + RB)], F16, tag="feat")
            nc.scalar.activation(out=feat[:], in_=rfr[:], func=AF.Sin,
                                 scale=float(2 * PI))
            uf = ufp.tile([128, 2 * RB], F16, tag="uf")
            nc.vector.tensor_scalar(
                out=uf[:, 0:RB], in0=feat[:, N:N + RB],
                scalar1=fbias_ap(2, toff), scalar2=None, op0=ALU.mult)
            nc.vector.tensor_scalar(
                out=uf[:, RB:], in0=feat[:, 2 * N + RB:],
                scalar1=fbias_ap(3, toff), scalar2=None, op0=ALU.mult)
            nc.tensor.matmul(lp[:], uf[:, 0:RB], feat[:, 0:N],
                             start=first[0], stop=False,
                             skip_group_check=True)
            first[0] = False
            nc.tensor.matmul(lp[:], uf[:, RB:], feat[:, N + RB:2 * N + RB],
                             start=False, stop=last, skip_group_check=True)

        # all pure-q readers first, then the g writes, then per-mod B/C
        # with the next mod's stage-B matmuls interleaved into C's PE idle.
        emit_relu3(0)
        for mi in range(3):
            nc.vector.tensor_mul(g_ap(mi), thl[mi][:],
                                 q_ap(mi).bitcast(F32))

        psUV0 = psuv.tile([114, N], F32, tag="psUV")
        for c in range(3):
            emit_B_chunk(0, c, psUV0)
        emit_B_fin(0, psUV0)
        emit_relu3(1)

        toff = 0
        for mi in range(3):
            nxt = mi + 1
            pend = []
            if nxt < 3:
                psUVn = psuv.tile([114, N], F32, tag="psUV")
                pend = [lambda c=c: emit_B_chunk(nxt, c, psUVn)
                        for c in range(3)]
                pend.append(lambda: emit_B_fin(nxt, psUVn))
                if nxt < 2:
                    pend.append(lambda: emit_relu3(nxt + 1))
            for t in range(Ts[mi]):
                last = (mi == 2 and t == Ts[mi] - 1)
                iv = pend.pop(0) if pend else None
                emit_C_tile(mi, t, toff, last, interleave=iv)
                toff += 1
            for fn in pend:
                fn()

        # ---------- softmax: exp table, accumulated row-sum ----------
        _load_set(AF.Exp, AF.Tanh, AF.Relu, AF.Copy)
        nmx = sp.tile([RB, 1], F32, tag="nmx")
        nc.vector.reduce_max(out=nmx[:], in_=lp[:], axis=AX.X, negate=True)
        E16 = sp.tile([RB, N], F16, tag="E16")
        esum = sp.tile([RB, 1], F32, tag="esum")
        nc.scalar.activation(out=E16[:], in_=lp[:], func=AF.Exp,
                             bias=nmx[:, 0:1], accum_out=esum[:])
        rsi = sp.tile([RB, 1], F32, tag="rsi")
        nc.vector.reciprocal(rsi[:], esum[:])
        S16 = sp.tile([RB, N], F16, tag="S16")
        nc.vector.tensor_scalar(out=S16[:], in0=E16[:], scalar1=rsi[:, 0:1],
                                scalar2=None, op0=ALU.mult)

        # ---------- attention output: ta^T = target^T @ S^T (fp16) --------
        psta = psuv.tile([114, N], F32, tag="psUV")
        id48 = cf16[0:48, 96:144]
        for c in range(3):
            pst = pst16.tile([128, RB], F16, tag="pst")
            nc.tensor.transpose(pst[:], S16[:, 128 * c:128 * c + 128],
                                id48[:])
            stc = sp.tile([128, RB], F16, tag="stc")
            nc.scalar.copy(stc[:], pst[:])
            nc.tensor.matmul(psta[0:HD, 0:RB],
                             cf16[:, 32 * c:32 * c + 32], stc[:],
                             start=(c == 0), stop=(c == 2))

        # ---------- final 2 KAN layers, transposed layout.  The q-style
        # operands (cur, g) are fp32r (input-perturbation-safe); the relu^3
        # chunk matmuls stay fp32 (feature rounding fails numerically).
        curlg = sp.tile([64, RB], F32R, tag="curlg0")
        nc.scalar.copy(curlg[0:32, :], psta[0:HD, 0:RB])
        for li in range(2):
            psr4 = ps4.tile([128, N], F32, tag="psq4")
            nc.tensor.matmul(psr4[:, 0:RB], sel4r, curlg[0:32, :],
                             start=True, stop=True)
            lth = thp.tile([HD, RB], F32, tag="lth")
            nc.scalar.activation(out=lth[:], in_=curlg[0:32, :].bitcast(F32),
                                 func=AF.Tanh, scale=0.5)
            nc.vector.tensor_mul(curlg[32:64, :], lth[:],
                                 curlg[0:32, :].bitcast(F32))
            psl = psuv.tile([114, N], F32, tag="psUV")
            for c in range(3):
                lfb = lfp.tile([128, RB], F32, tag="lfb")
                nc.vector._custom_dve(RELU3, out=lfb[:], in0=psr4[:, 0:RB],
                                      s0=cfA[:, OF_BL + c:OF_BL + c + 1])
                nc.tensor.matmul(psl[0:HD, 0:RB],
                                 cfA[:, OF_WL + HD * (3 * li + c):
                                      OF_WL + HD * (3 * li + c + 1)],
                                 lfb[:], start=(c == 0), stop=False)
            nc.tensor.matmul(psl[0:HD, 0:RB], wlsil_ap(li), curlg[:],
                             start=False, stop=True)
            nxtc = sp.tile([64, RB], F32R, tag=f"curlg{li + 1}")
            nc.scalar.activation(out=nxtc[0:32, :], in_=psl[0:HD, 0:RB],
                                 func=AF.Relu)
            curlg = nxtc

        nc.sync.dma_start(out=dout[:], in_=curlg[0:32, :])

    nc.finalize()
    return nc


_CACHED = {}


def _get_program(Ts):
    key = tuple(Ts)
    if key not in _CACHED:
        _CACHED[key] = build_program(Ts)
    return _CACHED[key]


def _in_maps(inputs, consts):
    tabs = consts['tabs']
    Ts = [t['T'] for t in tabs]
    Tsum = max(sum(Ts), 1)
    selw = _r11(np.concatenate([t['selw'] for t in tabs], axis=1))
    fbias = np.zeros((128, 4 * Tsum), np.float32)
    off = 0
    for t in tabs:
        T = t['T']
        fbias[:, off:off + T] = t['biasVc'][:, 0:T]
        fbias[:, Tsum + off:Tsum + off + T] = t['biasVs'][:, 0:T]
        fbias[:, 2 * Tsum + off:2 * Tsum + off + T] = t['rsC'][:, 0:T]
        fbias[:, 3 * Tsum + off:3 * Tsum + off + T] = t['rsS'][:, 0:T]
        off += T

    CA = 128 + 3 * 114 + 2 * HD
    OF_WS = 128
    OF_LS = 128 + 3 * 114
    crA = np.zeros((64, CA), np.float32)
    crA[0:32, 0:128] = consts['sel4']
    for mi in range(3):
        crA[:, OF_WS + 114 * mi:OF_WS + 114 * (mi + 1)] = consts['wsil'][mi]
    for li in range(2):
        crA[:, OF_LS + HD * li:OF_LS + HD * (li + 1)] = \
            consts['wlsil'][:, li, :]

    crB = np.zeros((128, 3 * 114), np.float32)
    for mi in range(3):
        crB[:, 114 * mi:114 * (mi + 1)] = consts['w1p32'][mi][:, 2, :]

    OF_FB = 0
    OF_BL = 4 * Tsum
    OF_WL = OF_BL + 3
    CFA = OF_WL + 6 * HD
    cfA = np.zeros((128, CFA), np.float32)
    cfA[:, OF_FB:OF_FB + 4 * Tsum] = fbias
    cfA[:, OF_BL:OF_BL + 3] = consts['biasl']
    for j in range(6):
        cfA[:, OF_WL + HD * j:OF_WL + HD * (j + 1)] = consts['wl32'][:, j, :]

    w1pB = np.zeros((128, 6 * 114), np.float32)
    for mi in range(3):
        for c in range(2):
            w1pB[:, (2 * mi + c) * 114:(2 * mi + c + 1) * 114] = \
                consts['w1p32'][mi][:, c, :]

    x, y, t = (np.ascontiguousarray(inputs[k], dtype=np.float32)
               for k in ('x', 'y', 'target'))
    maps = []
    for c in range(NCORES):
        xr = np.roll(x, -RB * c, axis=0)
        yr = np.roll(y, -RB * c, axis=0)
        tr = np.roll(t, -RB * c, axis=0)
        qT = np.concatenate([xr.T, yr.T, tr.T], axis=1)   # [32, 3*384]
        cf16 = np.zeros((128, 96 + 48), np.float16)
        cf16[:, 0:96] = tr.astype(np.float16).reshape(3, 128, HD) \
            .transpose(1, 0, 2).reshape(128, 96)
        cf16[0:48, 96:144] = consts['id48_16']
        qTr = _r11(qT)
        m = {'crA': _r11(crA), 'crB': _r11(crB), 'qT': qTr,
             'q4r': np.tile(qTr, (4, 1)),
             'selw': selw, 'cfA': cfA, 'w1pB': w1pB, 'cf16': cf16}
        maps.append(m)
    return maps, Ts


def kernel(**inputs) -> np.ndarray:
    from concourse.bass_utils import run_bass_kernel_spmd
    consts = _prepare(inputs)
    maps, Ts = _in_maps(inputs, consts)
    nc = _get_program(Ts)
    res = run_bass_kernel_spmd(nc, maps, core_ids=list(range(NCORES)))
    out = np.concatenate([res.results[c]['outT'].T for c in range(NCORES)],
                         axis=0)
    return out.astype(np.float32)


if __name__ == '__main__':
    import reference as ref
    inputs = {k: np.asarray(v) for k, v in ref.setup_inputs().items()}
    consts = _prepare(inputs)
    maps, Ts = _in_maps(inputs, consts)
    print("Ts =", Ts, "rows =", [t['K'] for t in consts['tabs']])
    nc = _get_program(Ts)
    print("program built ok")


# revision 14
# speedup vs baseline: 1.0345x; 1.0345x over previous
"""Trainium2 Bass kernel for nn_CrossModalAttention (KAN cross-modal attention).

v3 — restructured from the 88us v2 baseline (PE-bound, all-fp32 matmuls):

Math (same factorizations as v2):
  1. Pairwise KAN layer-1 separates: z_ij = U[i] + V[j]; U/V computed with the
     truncated-power cubic form (relu^3 shifts + silu-via-tanh) as matmuls.
  2. Pairwise layer-2 scalar KAN via a trimmed Fourier fit
       A = sum_k R_k cos(om_k (U+V) - ph_k)
     expanded by the cosine addition theorem into rank-2K fp16 matmuls.
     NEW: the phase is split symmetrically (ph/2 on the U side and V side),
     so the cos- and sin-feature blocks each share ONE per-row bias across
     both U and V columns -> 2 FRAC ops per tile instead of 4, and the
     sign of the sin-product is folded into a negated amplitude table.
  3. Fusion softmax weights w[3] evaluated on host, folded into amplitudes.

Performance structure vs v2:
  - float32r (11-bit-mantissa single-pass PE mode, 1 cyc/col) for every
    matmul where operand rounding is provably benign: q-replication, silu,
    phase matmuls.  Producers write float32r-typed tiles (DMA'd tables are
    pre-rounded on host).  The relu^3 chunk matmuls stay fp32: their
    truncated-power features cancel catastrophically under 11-bit rounding
    (measured 1.6e-2 vs 5.4e-3 final error). Final KAN layers stay fp32.
  - silu matmul pair merged into one 64-contraction matmul ([q; q*tanh(q/2)]
    stacked in partitions).
  - Fourier tile budget trimmed 13 -> 10 tiles with exact-128-row blocks
    (greedy per-mod allocation; measured 8.8e-3 final error).
  - amplitude scales moved to the idle GpSimd engine.
  - softmax via ACT Exp (+accumulated row-sum in the same instruction)
    instead of the tanh/reciprocal chain; exp_and_others table also serves
    the final KAN's tanh/relu so only one table switch happens.
  - consts packed into 6 DMA descriptors split across the SP/ACT queues.

Sharding: row-parallel over 8 cores, np.roll'd inputs, identical SPMD
program computes rows [0:48) of its rolled view; host concatenates.
"""
import math
from math import comb

import numpy as np

import concourse.bass as bass
import concourse.bacc as bacc
import concourse.mybir as mybir
import concourse.tile as tile

F32 = mybir.dt.float32
F32R = mybir.dt.float32r
F16 = mybir.dt.float16
AF = mybir.ActivationFunctionType
ALU = mybir.AluOpType
AX = mybir.AxisListType
PI = math.pi

# ---- problem constants (hardcoded from the nn.Module spec) ----
N, HD, MH = 384, 32, 50          # seq len, head dim, KAN hidden width
NCORES = 8
RB = N // NCORES                 # 48 output rows per core
GH = 0.4                         # knot spacing
GRID = np.arange(-3, 9) * GH - 1.0   # 12 knots -2.2 .. 2.2
MM = 16                          # Fourier modes per feature
TILE_BUDGET = 10                 # 128-row Fourier tiles across modalities
MARGIN, SLACK = 0.35, 1.5        # fit range margin / period slack
MAGIC = 12582912.0               # 1.5 * 2^23 fp32 round-to-int magic

# truncated-power -> B-spline conversion kappa[b, k]
KAPPA = np.zeros((8, 12), np.float64)
for b in range(8):
    for s in range(5):
        KAPPA[b, b + s] = (-1) ** s * comb(4, s) / (6 * GH ** 3)


def _r11(x):
    """Round fp32 to 11 explicit mantissa bits (the float32r encoding)."""
    x = np.ascontiguousarray(x, np.float32)
    u = x.view(np.uint32).astype(np.uint64)
    lsb = (u >> 12) & 1
    u = (u + 0x7FF + lsb) & np.uint64(0xFFFFF000)
    return u.astype(np.uint32).view(np.float32)


# ======================= custom DVE micro-ops =======================

_CUSTOM = {}


def _register_custom_ops():
    if _CUSTOM:
        return _CUSTOM
    from concourse import dve_ops
    from concourse.dve_spec import Spec, Src0, C0, C1, lower, _has_src1, relu, sq
    from concourse.dve_uop import DveOpSpec

    def reg(name, body, reference):
        for o in dve_ops.OPS:
            if o.name == name:
                _CUSTOM[name] = o
                return
        spec = Spec(body=body, reference=reference)
        row = dve_ops._CUSTOM_DVE_ROW_BASE + len(dve_ops.OPS)
        shas = {v: DveOpSpec(name=name, opcode=row, uops=lower(spec, ver=v),
                             rd1_en=_has_src1(spec)).sha(v)
                for v in ("v3", "v4")}
        op = dve_ops.DveOp(name, spec, subdim=False, uops_sha=shas)
        dve_ops.OPS.append(op)
        dve_ops.CUSTOM_DVE_SPECS[name] = spec
        dve_ops._SUB_OPCODE_FOR_NAME[name] = row
        _CUSTOM[name] = op

    f32 = np.float32
    # out = y - round(y), y = in0 + c1 (phase bias; per-partition AP or imm),
    # via the fp32 magic-number constant c0
    _y = Src0 + C1

    def _frac_ref(in0, in1, s0, s1, imm2):
        y = (in0.astype(f32) + np.asarray(s1, f32)).astype(f32)
        return (y - ((y + f32(s0)) - f32(s0))).astype(f32)

    reg("FRAC_SHIFT_ANT", _y - ((_y + C0) - C0), _frac_ref)
    # out = relu(in0 + c0)^3  (c0 may be a per-partition AP: the -g_k shift)
    _r3 = lambda in0, in1, s0, s1, imm2: np.maximum(
        in0.astype(f32) + np.asarray(s0, f32), 0).astype(f32) ** 3
    _rshift = relu(Src0 + C0)
    reg("RELU3_SHIFT_ANT", sq(_rshift) * _rshift, _r3)
    # Dekker residual: lo = t - hi12(t), t = relu(in0+c0)^3, c2 = 2^12+1.
    # hi12 matches the fp32r write-convert (round-to-nearest on the
    # 12-significand-bit grid), so the hi (fp32r RELU3) + lo matmul pair
    # reconstructs t to ~23 bits while both operands are fp32r-legal.
    from concourse.dve_spec import C2 as _C2
    _t = sq(_rshift) * _rshift
    _u = _t * _C2
    _hi = _u - (_u - _t)

    def _lo_ref(in0, in1, s0, s1, imm2):
        t = np.maximum(in0.astype(f32) + np.asarray(s0, f32),
                       0).astype(f32) ** 3
        u = (t * f32(imm2)).astype(f32)
        hi = (u - (u - t).astype(f32)).astype(f32)
        return (t - hi).astype(f32)

    reg("RELU3_LO_ANT", _t - _hi, _lo_ref)
    return _CUSTOM


# ======================= host-side precompute =======================

def _silu(x):
    return x / (1.0 + np.exp(-x))


def _bsplines(x):
    xe = x[..., None]
    g = GRID
    bases = ((xe >= g[:-1]) & (xe < g[1:])).astype(np.float64)
    for k in range(1, 4):
        left = (xe - g[:-(k + 1)]) / (g[k:-1] - g[:-(k + 1)]) * bases[..., :-1]
        right = (g[k + 1:] - xe) / (g[k + 1:] - g[1:-k]) * bases[..., 1:]
        bases = left + right
    return bases


def _kan_linear_host(x, bw, sw):
    base = _silu(x) @ bw.T
    spl = _bsplines(x)
    return base + np.einsum('...ik,oik->...o', spl, sw)


def _kan_pack(bw, sw):
    """KAN layer (bw [O,I], sw [O,I,8]) -> truncated-power weights
    W [13*I, O]: blocks 0..11 = relu^3(x - g_k) coefs, block 12 = silu."""
    O, I = bw.shape
    d = np.einsum('oib,bk->oik', sw.astype(np.float64), KAPPA)
    W = np.zeros((13 * I, O), np.float64)
    for k in range(12):
        W[k * I:(k + 1) * I, :] = d[:, :, k].T
    W[12 * I:, :] = bw.T
    return W


def _layer1_UV_host(q, bw1, sw1):
    swL, swR = sw1[:, :HD, :], sw1[:, HD:, :]
    spl = _bsplines(q)
    U = _silu(q) @ bw1[:, :HD].T + np.einsum('nik,oik->no', spl, swL)
    V = _silu(q) @ bw1[:, HD:].T + np.einsum('nik,oik->no', spl, swR)
    return U, V


def _phi_eval(z, bw2, sw2):
    return bw2[0][None, :] * _silu(z)[:, None] + _bsplines(z) @ sw2[0].T


def _fit_mod(inp, pre, qn):
    """Initial unweighted Fourier fit (for row ranking) + fit context."""
    q = inp[qn].astype(np.float64)
    U, V = _layer1_UV_host(q, inp[pre + '1bw'], inp[pre + '1sw'])
    zlo = U.min() + V.min() - MARGIN
    zhi = U.max() + V.max() + MARGIN
    S = 4001
    t = np.linspace(zlo, zhi, S)
    targ = _phi_eval(t, inp[pre + '2bw'], inp[pre + '2sw'])
    P = (zhi - zlo) + SLACK
    om = 2 * PI * np.arange(1, MM + 1) / P
    A = np.concatenate([np.ones((S, 1)),
                        np.cos(t[:, None] * om[None, :]),
                        np.sin(t[:, None] * om[None, :])], axis=1)
    coef, *_ = np.linalg.lstsq(A, targ, rcond=None)
    a, b = coef[1:MM + 1].T, coef[MM + 1:].T
    R = np.hypot(a, b)
    dens = []
    for f in range(MH):
        z = (U[:, f][:, None] + V[:, f][None, :]).ravel()
        h, edges = np.histogram(z, bins=160, range=(zlo, zhi))
        dens.append((0.5 * (edges[:-1] + edges[1:]), h.astype(np.float64)))
    return dict(U=U, V=V, om=om, R=R, t=t, targ=targ, A=A, dens=dens)


def _refit_rows(fit, keep_mask):
    """Per-feature weighted refit using only kept modes."""
    t, targ, A = fit['t'], fit['targ'], fit['A']
    R = np.zeros((MH, MM))
    ph = np.zeros((MH, MM))
    for f in range(MH):
        idxs = [m for m in range(MM) if keep_mask[f, m]]
        if not idxs:
            continue
        cols = [0] + [1 + m for m in idxs] + [1 + MM + m for m in idxs]
        xs, hs = fit['dens'][f]
        w = np.interp(t, xs, hs)
        w = np.sqrt(w / max(w.max(), 1e-12) + 1e-3)
        Af = A[:, cols] * w[:, None]
        coef, *_ = np.linalg.lstsq(Af, targ[:, f] * w, rcond=None)
        nk = len(idxs)
        a, b = coef[1:1 + nk], coef[1 + nk:]
        R[f, idxs] = np.hypot(a, b)
        ph[f, idxs] = np.arctan2(b, a)
    return R, ph


def _prepare(inputs):
    """All host precompute: fusion weights, fits, trimming, packed consts."""
    inp = {k: np.asarray(v) for k, v in inputs.items()}

    # ---- fusion weights w3 (exact, host) ----
    feats = np.concatenate([inp['x'].mean(0), inp['y'].mean(0),
                            inp['target'].mean(0)]).astype(np.float64)[None, :]
    h1 = _kan_linear_host(feats, inp['f1bw'].astype(np.float64),
                          inp['f1sw'].astype(np.float64))
    h2 = _kan_linear_host(h1, inp['f2bw'].astype(np.float64),
                          inp['f2sw'].astype(np.float64))[0]
    e = np.exp(h2 - h2.max())
    w3 = e / e.sum()

    mods = [('x', 'x'), ('y', 'y'), ('t', 'target')]
    fits = [_fit_mod(inp, pre, qn) for pre, qn in mods]

    # ---- greedy 128-row-block allocation of TILE_BUDGET tiles ----
    sorted_scores = []
    for mi, fit in enumerate(fits):
        s = np.sort((w3[mi] * fit['R']).ravel())[::-1]
        sorted_scores.append(s)
    tcount = [0, 0, 0]
    for _ in range(TILE_BUDGET):
        best, bi = -1.0, 0
        for mi in range(3):
            lo = 128 * tcount[mi]
            if lo >= MH * MM:
                continue
            v = sorted_scores[mi][lo:lo + 128].sum()
            if v > best:
                best, bi = v, mi
        tcount[bi] += 1
    keep_masks = []
    for mi, fit in enumerate(fits):
        k = min(128 * tcount[mi], MH * MM)
        sc = (w3[mi] * fit['R']).ravel()
        thr_idx = np.argsort(sc)[::-1][:k]
        km = np.zeros(MH * MM, bool)
        km[thr_idx] = True
        keep_masks.append(km.reshape(MH, MM))

    # ---- per-mod device tables (ph/2-split biases, signed amplitudes) ----
    tabs = []
    for mi, fit in enumerate(fits):
        km = keep_masks[mi]
        R2, ph2 = _refit_rows(fit, km)
        rows = [(f, m) for f in range(MH) for m in range(MM) if km[f, m]]
        K = len(rows)
        T = max(tcount[mi], 1)
        omr = np.array([fit['om'][m] for f, m in rows]) / (2 * PI)
        pr = np.array([ph2[f, m] for f, m in rows]) / (2 * PI)
        Rr = np.array([R2[f, m] for f, m in rows]) * w3[mi]
        fsel = np.array([f for f, m in rows], np.int64)
        selw = np.zeros((MH, T * 128), np.float32)
        selw[fsel, np.arange(K)] = omr
        biasVc = np.zeros((128, T), np.float32)
        biasVs = np.zeros((128, T), np.float32)
        rsC = np.zeros((128, T), np.float32)
        rsS = np.zeros((128, T), np.float32)
        for r in range(K):
            t_, p_ = divmod(r, 128)
            biasVc[p_, t_] = -pr[r] / 2 + 0.25
            biasVs[p_, t_] = -pr[r] / 2 + 0.5
            rsC[p_, t_] = Rr[r]
            rsS[p_, t_] = -Rr[r]
        tabs.append(dict(K=K, T=T, selw=selw, biasVc=biasVc, biasVs=biasVs,
                         rsC=rsC, rsS=rsS))

    # ---- layer-1 packed weights ----
    # chunk weights stay fp32 (cancellation-sensitive); silu chunk merged
    # [q; g] -> one 64-contraction fp32r matmul.
    w1p32 = []   # per mod [128, 3, 114] fp32  (relu^3 chunks 0..2)
    wsil = []    # per mod [64, 114] fp32r     (0.5 * silu chunk, stacked x2)
    for pre, qn in mods:
        bw1, sw1 = inp[pre + '1bw'], inp[pre + '1sw']
        WL = _kan_pack(bw1[:, :HD], sw1[:, :HD, :])     # [416, 50]
        WR = _kan_pack(bw1[:, HD:], sw1[:, HD:, :])
        Wb = np.zeros((416, 114))
        Wb[:, 0:MH] = WL
        Wb[:, 64:64 + MH] = WR
        ch = Wb.reshape(13, 32, 114)
        full = np.zeros((128, 3, 114))
        for c in range(3):
            full[:, c, :] = ch[4 * c:4 * c + 4].reshape(128, 114)
        w1p32.append(full.astype(np.float32))
        ws = np.zeros((64, 114), np.float32)
        ws[0:32] = 0.5 * ch[12]
        ws[32:64] = 0.5 * ch[12]
        wsil.append(_r11(ws))

    # ---- l-KAN packed weights (fp32 end to end) ----
    wl32 = np.zeros((128, 6, HD), np.float32)   # l1 chunks 0..2, l2 3..5
    wlsil = np.zeros((64, 2, HD), np.float32)
    for li, lname in enumerate(('l1', 'l2')):
        W = _kan_pack(inp[lname + 'bw'], inp[lname + 'sw'])  # [13*32, 32]
        ch = W.reshape(13, 32, HD)
        for c in range(3):
            wl32[:, 3 * li + c, :] = ch[4 * c:4 * c + 4].reshape(128, HD)
        wlsil[0:32, li, :] = 0.5 * ch[12]
        wlsil[32:64, li, :] = 0.5 * ch[12]

    # ---- misc consts ----
    sel4 = np.zeros((HD, 128), np.float32)
    for r in range(128):
        sel4[r % 32, r] = 1.0
    biasl = np.zeros((128, 3), np.float32)
    for c in range(3):
        for p in range(128):
            biasl[p, c] = -GRID[4 * c + p // 32]
    id48_16 = np.eye(48, dtype=np.float16)

    consts = dict(w3=w3, tabs=tabs, w1p32=w1p32, wsil=wsil, wl32=wl32,
                  wlsil=wlsil, sel4=sel4, biasl=biasl, id48_16=id48_16)
    return consts


# ======================= device program =======================

def build_program(Ts):
    """Ts = (T_x, T_y, T_t) tile counts."""
    ops = _register_custom_ops()
    FRAC, RELU3 = ops["FRAC_SHIFT_ANT"], ops["RELU3_SHIFT_ANT"]
    RELU3LO = ops["RELU3_LO_ANT"]
    nc = bacc.Bacc(None, target_bir_lowering=False)
    Tsum = sum(Ts)

    # DRAM inputs.
    # crA (fp32r): sel4 [32,128] | wsil x3 [64,114] | wlsil [64,64]
    # crB (fp32r): w1p chunk-2 weights x3 [128,114] (bounded features)
    # qT  (fp32r): [32, 3*384] rolled q for the 3 modalities
    # selw(fp32r): [50, Tsum*128]
    # cfA (fp32):  fbias [128,4*Tsum] | biasl [128,3] | wl32 [128,192]
    # w1pB (fp32): chunk-0/1 weights x3 [128, 6*114]
    # cf16 (fp16): tnat16 [128, 96] | id48 rows 0:48 cols 96:144
    CA = 128 + 3 * 114 + 2 * HD     # 534
    OF_WS = 128
    OF_LS = 128 + 3 * 114
    OF_FB = 0
    OF_BL = 4 * Tsum
    OF_WL = OF_BL + 3
    CFA = OF_WL + 6 * HD
    din = {}
    for nm, shp, dt in [
            ('crA', [64, CA], F32R),
            ('crB', [128, 3 * 114], F32R),
            ('qT', [32, 3 * N], F32R),
            ('selw', [MH, Tsum * 128], F32R),
            ('cfA', [128, CFA], F32),
            ('w1pB', [128, 6 * 114], F32),
            ('cf16', [128, 96 + 48], F16),
    ]:
        din[nm] = nc.dram_tensor(nm, shp, dt, kind="ExternalInput")
    dout = nc.dram_tensor("outT", [HD, RB], F32R, kind="ExternalOutput")

    with tile.TileContext(nc) as tc, \
         tc.tile_pool(name="consts", bufs=1) as cp, \
         tc.tile_pool(name="thp", bufs=3) as thp, \
         tc.tile_pool(name="fbp", bufs=6) as fbp, \
         tc.tile_pool(name="uvp", bufs=3) as uvp, \
         tc.tile_pool(name="frp", bufs=3) as frp, \
         tc.tile_pool(name="ftp", bufs=3) as ftp, \
         tc.tile_pool(name="ufp", bufs=3) as ufp, \
         tc.tile_pool(name="sp", bufs=2) as sp, \
         tc.tile_pool(name="lfp", bufs=3) as lfp, \
         tc.tile_pool(name="ps4", bufs=2, space="PSUM") as ps4, \
         tc.tile_pool(name="psuv", bufs=1, space="PSUM") as psuv, \
         tc.tile_pool(name="psph", bufs=3, space="PSUM") as psph, \
         tc.tile_pool(name="pslp", bufs=1, space="PSUM") as pslp, \
         tc.tile_pool(name="pst16", bufs=1, space="PSUM") as pst16:

        # ---- const loads: descriptors split across SP / ACT hwdge queues,
        # ordered so the critical path (q -> tanh/sel4; biasl -> relu3;
        # chunk weights) unblocks earliest.
        crA = cp.tile([64, CA], F32R, tag="crA")
        crB = cp.tile([128, 3 * 114], F32R, tag="crB")
        qg = cp.tile([64, 3 * N], F32R, tag="qg")
        selw = cp.tile([MH, Tsum * 128], F32R, tag="selw")
        cfA = cp.tile([128, CFA], F32, tag="cfA")
        w1pB = cp.tile([128, 6 * 114], F32, tag="w1pB")
        cf16 = cp.tile([128, 96 + 48], F16, tag="cf16")
        nc.sync.dma_start(out=qg[0:32, :], in_=din['qT'][:])
        nc.scalar.dma_start(out=cfA[:], in_=din['cfA'][:])
        nc.sync.dma_start(out=w1pB[:], in_=din['w1pB'][:])
        nc.scalar.dma_start(out=crA[:], in_=din['crA'][:])
        nc.sync.dma_start(out=crB[:], in_=din['crB'][:])
        nc.scalar.dma_start(out=selw[:], in_=din['selw'][:])
        nc.sync.dma_start(out=cf16[:], in_=din['cf16'][:])

        sel4r = crA[0:32, 0:128]

        def wsil_ap(mi):
            return crA[0:64, OF_WS + 114 * mi:OF_WS + 114 * (mi + 1)]

        def wlsil_ap(li):
            return crA[0:64, OF_LS + HD * li:OF_LS + HD * (li + 1)]

        def w1p_ap(mi, c):
            if c == 2:
                return crB[:, 114 * mi:114 * (mi + 1)]
            c0 = (2 * mi + c) * 114
            return w1pB[:, c0:c0 + 114]

        def fbias_ap(kind, toff):
            # kind 0=biasVc 1=biasVs 2=rsC 3=rsS
            c0 = OF_FB + kind * Tsum + toff
            return cfA[:, c0:c0 + 1]

        def q_ap(mi):
            return qg[0:32, N * mi:N * (mi + 1)]

        def qg_ap(mi):
            return qg[0:64, N * mi:N * (mi + 1)]

        def g_ap(mi):
            return qg[32:64, N * mi:N * (mi + 1)]

        # silu tanh prep + q-replication BEFORE the g writes: all pure
        # q readers are emitted first so the tile-granular write of g
        # into qg[32:64] doesn't serialize them.
        thl = []
        for mi in range(3):
            th = thp.tile([HD, N], F32, tag="thq")
            nc.scalar.activation(out=th[:], in_=q_ap(mi).bitcast(F32),
                                 func=AF.Tanh, scale=0.5)
            thl.append(th)

        uvs = [None] * 3
        fbs = [None] * 3
        psq4s = [None] * 3

        def emit_sel4(mi):
            psq4 = ps4.tile([128, N], F32, tag="psq4")
            nc.tensor.matmul(psq4[:], sel4r, q_ap(mi), start=True, stop=True)
            psq4s[mi] = psq4

        def emit_relu3(mi):
            # chunk 2's bounded features take fp32r rounding for free; the
            # big-magnitude chunks 0/1 stay fp32 (cancellation-sensitive).
            fbt = []
            for c in range(3):
                fb = fbp.tile([128, N], F32R if c == 2 else F32, tag="fb")
                nc.vector._custom_dve(RELU3, out=fb[:], in0=psq4s[mi][:],
                                      s0=cfA[:, OF_BL + c:OF_BL + c + 1])
                fbt.append((fb, None))
            fbs[mi] = fbt

        def emit_B_chunk(mi, c, psUV):
            hi, lo = fbs[mi][c]
            nc.tensor.matmul(psUV[:], w1p_ap(mi, c), hi[:],
                             start=(c == 0), stop=False)
            if lo is not None:
                nc.tensor.matmul(psUV[:], w1p_ap(mi, c), lo[:],
                                 start=False, stop=False)

        def emit_B_fin(mi, psUV):
            nc.tensor.matmul(psUV[:], wsil_ap(mi), qg_ap(mi),
                             start=False, stop=True)
            uv = uvp.tile([MH, N + RB], F32R, tag="uv")
            nc.scalar.copy(uv[:, 0:N], psUV[64:64 + MH, :])       # V block
            nc.vector.tensor_copy(uv[:, N:N + RB], psUV[0:MH, 0:RB])  # U
            uvs[mi] = uv

        lp = pslp.tile([RB, N], F32, tag="lp")
        first = [True]

        def emit_C_tile(mi, t, toff, last, interleave=None):
            psPH = psph.tile([128, N + RB], F32, tag="psPH")
            nc.tensor.matmul(psPH[:],
                             selw[:, 128 * toff:128 * toff + 128],
                             uvs[mi][:], start=True, stop=True)
            if interleave is not None:
                interleave()
            rfr = frp.tile([128, 2 * (N + RB)], F32, tag="rfr")
            nc.vector._custom_dve(
                FRAC, out=rfr[:, 0:N + RB], in0=psPH[:], s0=MAGIC,
                s1=fbias_ap(0, toff))
            nc.vector._custom_dve(
                FRAC, out=rfr[:, N + RB:], in0=psPH[:], s0=MAGIC,
                s1=fbias_ap(1, toff))
            feat = ftp.tile([128, 2 * (N + RB)], F16, tag="feat")
            nc.scalar.activation(out=feat[:], in_=rfr[:], func=AF.Sin,
                                 scale=float(2 * PI))
            uf = ufp.tile([128, 2 * RB], F16, tag="uf")
            nc.vector.tensor_scalar(
                out=uf[:, 0:RB], in0=feat[:, N:N + RB],
                scalar1=fbias_ap(2, toff), scalar2=None, op0=ALU.mult)
            nc.vector.tensor_scalar(
                out=uf[:, RB:], in0=feat[:, 2 * N + RB:],
                scalar1=fbias_ap(3, toff), scalar2=None, op0=ALU.mult)
            nc.tensor.matmul(lp[:], uf[:, 0:RB], feat[:, 0:N],
                             start=first[0], stop=False,
                             skip_group_check=True)
            first[0] = False
            nc.tensor.matmul(lp[:], uf[:, RB:], feat[:, N + RB:2 * N + RB],
                             start=False, stop=last, skip_group_check=True)

        # all pure-q readers first, then the g writes, then per-mod B/C
        # with the next mod's stage-B matmuls interleaved into C's PE idle.
        emit_sel4(0)
        emit_sel4(1)
        emit_relu3(0)
        for mi in range(3):
            nc.vector.tensor_mul(g_ap(mi), thl[mi][:],
                                 q_ap(mi).bitcast(F32))

        psUV0 = psuv.tile([114, N], F32, tag="psUV")
        for c in range(3):
            emit_B_chunk(0, c, psUV0)
        emit_B_fin(0, psUV0)
        emit_relu3(1)

        toff = 0
        for mi in range(3):
            nxt = mi + 1
            pend = []
            if nxt < 3:
                psUVn = psuv.tile([114, N], F32, tag="psUV")
                pend = [lambda c=c: emit_B_chunk(nxt, c, psUVn)
                        for c in range(3)]
                pend.append(lambda: emit_B_fin(nxt, psUVn))
                if nxt < 2:
                    pend.append(lambda: (emit_sel4(nxt + 1),
                                         emit_relu3(nxt + 1)))
            for t in range(Ts[mi]):
                last = (mi == 2 and t == Ts[mi] - 1)
                iv = pend.pop(0) if pend else None
                emit_C_tile(mi, t, toff, last, interleave=iv)
                toff += 1
            for fn in pend:
                fn()

        # ---------- softmax: exp table, accumulated row-sum ----------
        nmx = sp.tile([RB, 1], F32, tag="nmx")
        nc.vector.reduce_max(out=nmx[:], in_=lp[:], axis=AX.X, negate=True)
        E16 = sp.tile([RB, N], F16, tag="E16")
        esum = sp.tile([RB, 1], F32, tag="esum")
        nc.scalar.activation(out=E16[:], in_=lp[:], func=AF.Exp,
                             bias=nmx[:, 0:1], accum_out=esum[:])
        rsi = sp.tile([RB, 1], F32, tag="rsi")
        nc.vector.reciprocal(rsi[:], esum[:])
        S16 = sp.tile([RB, N], F16, tag="S16")
        nc.vector.tensor_scalar(out=S16[:], in0=E16[:], scalar1=rsi[:, 0:1],
                                scalar2=None, op0=ALU.mult)

        # ---------- attention output: ta^T = target^T @ S^T (fp16) --------
        psta = psuv.tile([114, N], F32, tag="psUV")
        id48 = cf16[0:48, 96:144]
        for c in range(3):
            pst = pst16.tile([128, RB], F16, tag="pst")
            nc.tensor.transpose(pst[:], S16[:, 128 * c:128 * c + 128],
                                id48[:])
            stc = sp.tile([128, RB], F16, tag="stc")
            nc.scalar.copy(stc[:], pst[:])
            nc.tensor.matmul(psta[0:HD, 0:RB],
                             cf16[:, 32 * c:32 * c + 32], stc[:],
                             start=(c == 0), stop=(c == 2))

        # ---------- final 2 KAN layers, transposed layout.  The q-style
        # operands (cur, g) are fp32r (input-perturbation-safe); the relu^3
        # chunk matmuls stay fp32 (feature rounding fails numerically).
        curlg = sp.tile([64, RB], F32R, tag="curlg0")
        nc.scalar.copy(curlg[0:32, :], psta[0:HD, 0:RB])
        for li in range(2):
            psr4 = ps4.tile([128, N], F32, tag="psq4")
            nc.tensor.matmul(psr4[:, 0:RB], sel4r, curlg[0:32, :],
                             start=True, stop=True)
            lth = thp.tile([HD, RB], F32, tag="lth")
            nc.scalar.activation(out=lth[:], in_=curlg[0:32, :].bitcast(F32),
                                 func=AF.Tanh, scale=0.5)
            nc.vector.tensor_mul(curlg[32:64, :], lth[:],
                                 curlg[0:32, :].bitcast(F32))
            psl = psuv.tile([114, N], F32, tag="psUV")
            for c in range(3):
                lfb = lfp.tile([128, RB], F32, tag="lfb")
                nc.vector._custom_dve(RELU3, out=lfb[:], in0=psr4[:, 0:RB],
                                      s0=cfA[:, OF_BL + c:OF_BL + c + 1])
                nc.tensor.matmul(psl[0:HD, 0:RB],
                                 cfA[:, OF_WL + HD * (3 * li + c):
                                      OF_WL + HD * (3 * li + c + 1)],
                                 lfb[:], start=(c == 0), stop=False)
            nc.tensor.matmul(psl[0:HD, 0:RB], wlsil_ap(li), curlg[:],
                             start=False, stop=True)
            nxtc = sp.tile([64, RB], F32R, tag=f"curlg{li + 1}")
            nc.scalar.activation(out=nxtc[0:32, :], in_=psl[0:HD, 0:RB],
                                 func=AF.Relu)
            curlg = nxtc

        nc.sync.dma_start(out=dout[:], in_=curlg[0:32, :])

    nc.finalize()
    return nc


_CACHED = {}


def _get_program(Ts):
    key = tuple(Ts)
    if key not in _CACHED:
        _CACHED[key] = build_program(Ts)
    return _CACHED[key]


def _in_maps(inputs, consts):
    tabs = consts['tabs']
    Ts = [t['T'] for t in tabs]
    Tsum = max(sum(Ts), 1)
    selw = _r11(np.concatenate([t['selw'] for t in tabs], axis=1))
    fbias = np.zeros((128, 4 * Tsum), np.float32)
    off = 0
    for t in tabs:
        T = t['T']
        fbias[:, off:off + T] = t['biasVc'][:, 0:T]
        fbias[:, Tsum + off:Tsum + off + T] = t['biasVs'][:, 0:T]
        fbias[:, 2 * Tsum + off:2 * Tsum + off + T] = t['rsC'][:, 0:T]
        fbias[:, 3 * Tsum + off:3 * Tsum + off + T] = t['rsS'][:, 0:T]
        off += T

    CA = 128 + 3 * 114 + 2 * HD
    OF_WS = 128
    OF_LS = 128 + 3 * 114
    crA = np.zeros((64, CA), np.float32)
    crA[0:32, 0:128] = consts['sel4']
    for mi in range(3):
        crA[:, OF_WS + 114 * mi:OF_WS + 114 * (mi + 1)] = consts['wsil'][mi]
    for li in range(2):
        crA[:, OF_LS + HD * li:OF_LS + HD * (li + 1)] = \
            consts['wlsil'][:, li, :]

    crB = np.zeros((128, 3 * 114), np.float32)
    for mi in range(3):
        crB[:, 114 * mi:114 * (mi + 1)] = consts['w1p32'][mi][:, 2, :]

    OF_FB = 0
    OF_BL = 4 * Tsum
    OF_WL = OF_BL + 3
    CFA = OF_WL + 6 * HD
    cfA = np.zeros((128, CFA), np.float32)
    cfA[:, OF_FB:OF_FB + 4 * Tsum] = fbias
    cfA[:, OF_BL:OF_BL + 3] = consts['biasl']
    for j in range(6):
        cfA[:, OF_WL + HD * j:OF_WL + HD * (j + 1)] = consts['wl32'][:, j, :]

    w1pB = np.zeros((128, 6 * 114), np.float32)
    for mi in range(3):
        for c in range(2):
            w1pB[:, (2 * mi + c) * 114:(2 * mi + c + 1) * 114] = \
                consts['w1p32'][mi][:, c, :]

    x, y, t = (np.ascontiguousarray(inputs[k], dtype=np.float32)
               for k in ('x', 'y', 'target'))
    maps = []
    for c in range(NCORES):
        xr = np.roll(x, -RB * c, axis=0)
        yr = np.roll(y, -RB * c, axis=0)
        tr = np.roll(t, -RB * c, axis=0)
        qT = np.concatenate([xr.T, yr.T, tr.T], axis=1)   # [32, 3*384]
        cf16 = np.zeros((128, 96 + 48), np.float16)
        cf16[:, 0:96] = tr.astype(np.float16).reshape(3, 128, HD) \
            .transpose(1, 0, 2).reshape(128, 96)
        cf16[0:48, 96:144] = consts['id48_16']
        m = {'crA': _r11(crA), 'crB': _r11(crB), 'qT': _r11(qT),
             'selw': selw, 'cfA': cfA, 'w1pB': w1pB, 'cf16': cf16}
        maps.append(m)
    return maps, Ts


def kernel(**inputs) -> np.ndarray:
    from concourse.bass_utils import run_bass_kernel_spmd
    consts = _prepare(inputs)
    maps, Ts = _in_maps(inputs, consts)
    nc = _get_program(Ts)
    res = run_bass_kernel_spmd(nc, maps, core_ids=list(range(NCORES)))
    out = np.concatenate([res.results[c]['outT'].T for c in range(NCORES)],
                         axis=0)
    return out.astype(np.float32)


if __name__ == '__main__':
    import reference as ref
    inputs = {k: np.asarray(v) for k, v in ref.setup_inputs().items()}
    consts = _prepare(inputs)
    maps, Ts = _in_maps(inputs, consts)
    print("Ts =", Ts, "rows =", [t['K'] for t in consts['tabs']])
    nc = _get_program(Ts)
    print("program built ok")
